# revision 6
# baseline (speedup 1.0000x reference)
"""Trainium2 Bass kernel for CompositeLoss (0.16*MSE + 0.84*(1-SSIM)).

Data-parallel over 8 cores (2 images x 3 channels = 6 maps each). Per core,
per channel:
  - X,Y loaded as [128, 4, 512] (row r = partition + 128*jb)
  - pre-pass: U=X+Y, V=X-Y on Pool; UU=U^2 on DVE; VV=V^2 (+MSE accum) on ACT
  - stage-1 H-conv (fp16 PE): per (map, wc) 10 matmuls (4 owned-region +
    3 straddle pairs) into 2-bank PSUM pairs; evacuated to fp8 y1 via
    paired [128,2,502] copies split across ACT/DVE
  - stage-2 W-conv in fp8 DoubleRow (2 k-tiles per instr, 0.5 cyc/row):
    psum pairs [a|b], [h1|h2]
  - post-pass: P,Q = ACT square pair; B,A = Pool; n1d1 = DVE tensor_scalar;
    n2'd2' = DVE STT vs PSUM; NN,DD = DVE TT; recip on ACT; final
    accumulation on Pool STT
  - per-partition partial sums DMA'd out; host reduces + combines cores.
"""

import os
import sys

import numpy as np

sys.path.insert(0, "/opt/trn_rl_repo")

H = W = 512
OUT = 502
WIN = 11
SIG = 1.5
C1 = 0.01 ** 2
C2 = 0.03 ** 2
TC1 = float(2.0 * C1)
TC2 = float(2.0 * C2)
ALPHA = 0.84
NCH = 6
NCORES = 8
NACC = 32     # acc cols: 0..5 mse per ch, 8+ch*4+c ssim
SS0 = 8
OWN_OFF = [0, 128, 256, 384]


def _taps():
    c = np.arange(WIN, dtype=np.float64) - (WIN - 1) / 2.0
    g = np.exp(-(c ** 2) / (2.0 * SIG ** 2))
    g = g / g.sum()
    g16 = g.astype(np.float16).astype(np.float64)
    g16[5] = 1.0 - (g16.sum() - g16[5])
    g16 = g16.astype(np.float16).astype(np.float64)
    return g16


def _consts():
    import ml_dtypes
    g = _taps()
    f16 = np.float16
    fp8 = ml_dtypes.float8_e4m3

    band_own = np.zeros((128, 118), dtype=np.float64)
    for t in range(118):
        band_own[t:t + WIN, t] = g
    band_tail = np.zeros((128, 10), dtype=np.float64)
    for tl in range(10):
        for r in range(118 + tl, 128):
            band_tail[r, tl] = g[r - 118 - tl]
    band_head = np.zeros((128, 10), dtype=np.float64)
    for tl in range(10):
        for r in range(0, tl + 1):
            band_head[r, tl] = g[r + 10 - tl]

    bw = np.zeros((128, 128), dtype=np.float64)
    for m in range(128):
        k = np.arange(m, min(m + WIN, 128))
        bw[k, m] = g[k - m]
    bwh = np.zeros((128, 128), dtype=np.float64)
    for m in range(118, 128):
        k = np.arange(0, m - 118 + 1)
        bwh[k, m] = g[k + 128 - m]

    def renorm(mats, colsets):
        # nudge fp8 taps by one ulp each until every output column's tap
        # sum is 1 -- fp8 tap-sum error otherwise biases sigma estimates
        for locs in colsets:
            for _ in range(24):
                s = sum(float(mats[mi][r, c]) for mi, r, c in locs)
                err = 1.0 - s
                if abs(err) < 1e-7:
                    break
                best = None
                for mi, r, c in locs:
                    u = mats[mi][r, c].view(np.uint8)
                    for nb in (np.uint8(u + 1), np.uint8(u - 1)):
                        nv = nb.view(fp8)
                        nerr = abs(err - (float(nv) - float(mats[mi][r, c])))
                        if nerr < abs(err) - 1e-12 and (
                                best is None or nerr < best[0]):
                            best = (nerr, mi, r, c, nv)
                if best is None:
                    break
                _, mi, r, c, nv = best
                mats[mi][r, c] = nv
        return mats

    bw8, bwh8 = renorm(
        [bw.astype(fp8), bwh.astype(fp8)],
        [[(0, k, m) for k in range(m, min(m + WIN, 128))]
         + [(1, k, m) for k in range(0, max(0, m - 117))]
         for m in range(128)])
    bw118_8, = renorm(
        [bw[:, :118].astype(fp8)],
        [[(0, k, m) for k in range(m, m + WIN)] for m in range(118)])
    bwp8 = np.stack([bw8, bwh8], axis=1)       # [128, 2, 128]
    neg = lambda a: (a.view(np.uint8) ^ np.uint8(0x80)).view(fp8)

    return {
        "band_own": band_own.astype(f16),
        "band_tail": band_tail.astype(f16),
        "band_head": band_head.astype(f16),
        "bwp": bwp8,
        "bwpn": neg(bwp8),
        "bw118": bw118_8,
        "bw118n": neg(bw118_8),
    }


_NC_CACHE = {}

# evacuation engine per (map_idx, pair): "A" = ACT, "D" = DVE
EVAC = ["A", "D", "A", "D", "A", "A", "D", "A"]
# reciprocal engine per chunk c
RECIP = ["A", "D", "A", "D"]


def _build_nc():
    if "nc" in _NC_CACHE:
        return _NC_CACHE["nc"]
    from concourse import bass, bacc, mybir
    from concourse.tile import TileContext
    dt = mybir.dt
    AF = mybir.ActivationFunctionType
    OP = mybir.AluOpType
    DR = mybir.MatmulPerfMode.DoubleRow

    nc = bacc.Bacc(None, target_bir_lowering=False)
    pred = nc.dram_tensor("pred", [NCH, H, W], dt.float32, kind="ExternalInput")
    targ = nc.dram_tensor("targ", [NCH, H, W], dt.float32, kind="ExternalInput")
    bown_d = nc.dram_tensor("band_own", [128, 118], dt.float16, kind="ExternalInput")
    btail_d = nc.dram_tensor("band_tail", [128, 10], dt.float16, kind="ExternalInput")
    bhead_d = nc.dram_tensor("band_head", [128, 10], dt.float16, kind="ExternalInput")
    bwp_d = nc.dram_tensor("bwp", [128, 2, 128], dt.float8e4, kind="ExternalInput")
    bwpn_d = nc.dram_tensor("bwpn", [128, 2, 128], dt.float8e4, kind="ExternalInput")
    bw118_d = nc.dram_tensor("bw118", [128, 118], dt.float8e4, kind="ExternalInput")
    bw118n_d = nc.dram_tensor("bw118n", [128, 118], dt.float8e4, kind="ExternalInput")
    out_d = nc.dram_tensor("out_acc", [128, NACC], dt.float32, kind="ExternalOutput")

    with TileContext(nc) as tc:
        with (
            tc.tile_pool(name="const", bufs=1) as constp,
            tc.tile_pool(name="io", bufs=2) as iop,
            tc.tile_pool(name="maps", bufs=2) as mapp,
            tc.tile_pool(name="y1", bufs=2) as y1p,
            tc.tile_pool(name="post", bufs=2) as postp,
            tc.tile_pool(name="acc", bufs=1) as accp,
            tc.tile_pool(name="ps1", bufs=2, space="PSUM") as ps1p,
            tc.tile_pool(name="ps2", bufs=1, space="PSUM") as ps2p,
        ):
            bown = constp.tile([128, 118], dt.float16, name="bown")
            btail = constp.tile([128, 10], dt.float16, name="btail")
            bhead = constp.tile([128, 10], dt.float16, name="bhead")
            bwp = constp.tile([128, 2, 128], dt.float8e4, name="bwp")
            bwpn = constp.tile([128, 2, 128], dt.float8e4, name="bwpn")
            bw118 = constp.tile([128, 118], dt.float8e4, name="bw118")
            bw118n = constp.tile([128, 118], dt.float8e4, name="bw118n")
            nc.sync.dma_start(out=bown[:], in_=bown_d[:])
            nc.sync.dma_start(out=btail[:], in_=btail_d[:])
            nc.sync.dma_start(out=bhead[:], in_=bhead_d[:])
            nc.sync.dma_start(out=bwp[:], in_=bwp_d[:])
            nc.sync.dma_start(out=bwpn[:], in_=bwpn_d[:])
            nc.sync.dma_start(out=bw118[:], in_=bw118_d[:])
            nc.sync.dma_start(out=bw118n[:], in_=bw118n_d[:])

            acc = accp.tile([128, NACC], dt.float32, name="acc")
            nc.vector.memset(acc[:], 0.0)

            for ch in range(NCH):
                # ---- loads: [128, 4, 512], row r = p + 128*jb ----
                xt = iop.tile([128, 4, 512], dt.float32, tag="x", name=f"x{ch}")
                yt = iop.tile([128, 4, 512], dt.float32, tag="y", name=f"y{ch}")
                for jb in range(4):
                    r0 = 128 * jb
                    nc.sync.dma_start(out=xt[:, jb, :],
                                      in_=pred[ch, r0:r0 + 128, :])
                    nc.sync.dma_start(out=yt[:, jb, :],
                                      in_=targ[ch, r0:r0 + 128, :])

                # ---- pre-pass ----
                ut = mapp.tile([128, 4, 512], dt.float16, tag="u", name=f"u{ch}")
                vt = mapp.tile([128, 4, 512], dt.float16, tag="v", name=f"v{ch}")
                uut = mapp.tile([128, 4, 512], dt.float16, tag="uu", name=f"uu{ch}")
                vvt = mapp.tile([128, 4, 512], dt.float16, tag="vv", name=f"vv{ch}")
                nc.gpsimd.tensor_tensor(ut[:], xt[:], yt[:], OP.add)
                nc.gpsimd.tensor_tensor(vt[:], xt[:], yt[:], OP.subtract)
                nc.gpsimd.tensor_tensor(uut[:], ut[:], ut[:], OP.mult)
                # MSE = sum((X-Y)^2) rides the VV square
                nc.scalar.activation(vvt[:], vt[:], AF.Square,
                                     accum_out=acc[:, ch:ch + 1])

                maps_ = (ut, vt, uut, vvt)

                # ---- stage 1: H-conv (fp16) -> fp8 y1 [128, 16, 502] ----
                y1 = y1p.tile([128, 16, 502], dt.float8e4, tag="y1",
                              name=f"y1_{ch}")
                for mi in range(4):
                    mt = maps_[mi]
                    for pair in range(2):
                        ps = ps1p.tile([128, 2, 512], dt.float32, tag="ps1",
                                       name=f"ps1_{ch}_{mi}{pair}")
                        for half in range(2):
                            wc = pair * 2 + half
                            ws = slice(wc * 128, (wc + 1) * 128)
                            seq = []
                            for jb in range(4):
                                seq.append((mt[:, jb, ws], bown,
                                            OWN_OFF[jb], 118))
                            for b in (1, 2, 3):
                                seq.append((mt[:, b - 1, ws], btail,
                                            128 * b - 10, 10))
                                seq.append((mt[:, b, ws], bhead,
                                            128 * b - 10, 10))
                            n_mm = len(seq)
                            for i, (lhs, band, lo, n) in enumerate(seq):
                                nc.tensor.matmul(
                                    ps[:, half, lo:lo + n],
                                    lhsT=lhs, rhs=band[:, 0:n],
                                    start=(i == 0), stop=(i == n_mm - 1),
                                    skip_group_check=True)
                        dst = y1[:, mi * 4 + pair * 2: mi * 4 + pair * 2 + 2, :]
                        if EVAC[mi * 2 + pair] == "A":
                            nc.scalar.copy(dst, ps[:, :, 0:502])
                        else:
                            nc.vector.tensor_copy(dst, ps[:, :, 0:502])

                # ---- stage 2 (fp8 DoubleRow W-conv) + post-pass per c ----
                for c in range(4):
                    P = 128 if c < 3 else 118
                    ab = ps2p.tile([128, 2, 512], dt.float32, tag="ab",
                                   name=f"ab{ch}{c}")
                    hh = ps2p.tile([128, 2, 512], dt.float32, tag="hh",
                                   name=f"hh{ch}{c}")
                    if c < 3:
                        pr = lambda mi: y1[:, mi * 4 + c: mi * 4 + c + 2, :]
                        nc.tensor.matmul(ab[:, 0, 0:502], lhsT=bwp[:],
                                         rhs=pr(0), start=True, stop=True,
                                         perf_mode=DR, skip_group_check=True)
                        nc.tensor.matmul(ab[:, 1, 0:502], lhsT=bwp[:],
                                         rhs=pr(1), start=True, stop=True,
                                         perf_mode=DR, skip_group_check=True)
                        nc.tensor.matmul(hh[:, 0, 0:502], lhsT=bwp[:],
                                         rhs=pr(2), start=True, stop=False,
                                         perf_mode=DR, skip_group_check=True)
                        nc.tensor.matmul(hh[:, 0, 0:502], lhsT=bwpn[:],
                                         rhs=pr(3), start=False, stop=True,
                                         perf_mode=DR, skip_group_check=True)
                        nc.tensor.matmul(hh[:, 1, 0:502], lhsT=bwp[:],
                                         rhs=pr(2), start=True, stop=False,
                                         perf_mode=DR, skip_group_check=True)
                        nc.tensor.matmul(hh[:, 1, 0:502], lhsT=bwp[:],
                                         rhs=pr(3), start=False, stop=True,
                                         perf_mode=DR, skip_group_check=True)
                    else:
                        sl = lambda mi: y1[:, mi * 4 + 3, :]
                        nc.tensor.matmul(ab[0:118, 0, 0:502], lhsT=bw118[:],
                                         rhs=sl(0), start=True, stop=True,
                                         skip_group_check=True)
                        nc.tensor.matmul(ab[0:118, 1, 0:502], lhsT=bw118[:],
                                         rhs=sl(1), start=True, stop=True,
                                         skip_group_check=True)
                        nc.tensor.matmul(hh[0:118, 0, 0:502], lhsT=bw118[:],
                                         rhs=sl(2), start=True, stop=False,
                                         skip_group_check=True)
                        nc.tensor.matmul(hh[0:118, 0, 0:502], lhsT=bw118n[:],
                                         rhs=sl(3), start=False, stop=True,
                                         skip_group_check=True)
                        nc.tensor.matmul(hh[0:118, 1, 0:502], lhsT=bw118[:],
                                         rhs=sl(2), start=True, stop=False,
                                         skip_group_check=True)
                        nc.tensor.matmul(hh[0:118, 1, 0:502], lhsT=bw118[:],
                                         rhs=sl(3), start=False, stop=True,
                                         skip_group_check=True)

                    # post-pass
                    pq = postp.tile([128, 1004], dt.float16, tag="pq",
                                    name=f"pq{ch}{c}")
                    ba = postp.tile([128, 1004], dt.float16, tag="ba",
                                    name=f"ba{ch}{c}")
                    nd = postp.tile([128, 1004], dt.float16, tag="nd",
                                    name=f"nd{ch}{c}")
                    n2d2 = postp.tile([128, 1004], dt.float16, tag="n2d2",
                                      name=f"n2d2{ch}{c}")
                    nndd = postp.tile([128, 1004], dt.float16, tag="nndd",
                                      name=f"nndd{ch}{c}")
                    rr = postp.tile([128, 502], dt.float16, tag="rr",
                                    name=f"rr{ch}{c}")
                    junk = postp.tile([128, 502], dt.float16, tag="junk",
                                      name=f"junk{ch}{c}")

                    # P|Q = square(a|b)
                    nc.scalar.activation(pq[0:P, :], ab[0:P, :, 0:502],
                                         AF.Square)
                    # B = P-Q ; A = P+Q   (Pool)
                    nc.gpsimd.tensor_tensor(ba[0:P, 0:502], pq[0:P, 0:502],
                                            pq[0:P, 502:1004], OP.subtract)
                    nc.gpsimd.tensor_tensor(ba[0:P, 502:1004], pq[0:P, 0:502],
                                            pq[0:P, 502:1004], OP.add)
                    # n1|d1 = (B|A) + 2C1
                    nc.vector.tensor_scalar_add(nd[0:P, :], ba[0:P, :], TC1)
                    # n2'|d2' = (B|A - 2C2) - (h1|h2)   [negated n2, d2]
                    nc.vector.scalar_tensor_tensor(
                        n2d2[0:P, :], ba[0:P, :], TC2, hh[0:P, :, 0:502],
                        OP.subtract, OP.subtract)
                    # NN|DD  (Pool)
                    nc.gpsimd.tensor_tensor(nndd[0:P, :], nd[0:P, :],
                                            n2d2[0:P, :], OP.mult)
                    with nc.allow_low_precision(reason="fp16 ssim recip"):
                        nc.vector.reciprocal(rr[0:P, :],
                                             nndd[0:P, 502:1004])
                        sidx = SS0 + ch * 4 + c
                        nc.vector.scalar_tensor_tensor(
                            junk[0:P, :], nndd[0:P, 0:502], 1.0, rr[0:P, :],
                            OP.mult, OP.mult,
                            accum_out=acc[0:P, sidx:sidx + 1])

            nc.sync.dma_start(out=out_d[:], in_=acc[:])

    nc.compile()
    _NC_CACHE["nc"] = nc
    return nc


def kernel(pred: np.ndarray, target: np.ndarray) -> np.ndarray:
    from concourse.bass_utils import run_bass_kernel_spmd

    pred = np.asarray(pred, dtype=np.float32)
    target = np.asarray(target, dtype=np.float32)
    cst = _consts()

    nc = _build_nc()
    in_maps = []
    for i in range(NCORES):
        m = {
            "pred": pred[2 * i:2 * i + 2].reshape(NCH, H, W),
            "targ": target[2 * i:2 * i + 2].reshape(NCH, H, W),
        }
        m.update(cst)
        in_maps.append(m)

    trace = os.environ.get("BASS_SSIM_TRACE", "0") == "1"
    res = run_bass_kernel_spmd(nc, in_maps, core_ids=list(range(NCORES)),
                               trace=trace)
    if trace and res.exec_time_ns is not None:
        print(f"HW exec time: {res.exec_time_ns} ns")
        _NC_CACHE["exec_time_ns"] = res.exec_time_ns

    mse_sum = 0.0
    ssim_sum = 0.0
    for i in range(NCORES):
        o = np.asarray(res.results[i]["out_acc"], dtype=np.float64)
        mse_sum += float(o[:, 0:NCH].sum())
        ssim_sum += float(o[:, SS0:SS0 + NCH * 4].sum())

    mse_mean = mse_sum / (16 * 3 * H * W)
    ssim_mean = ssim_sum / (16 * 3 * OUT * OUT)
    loss = (1.0 - ALPHA) * mse_mean + ALPHA * (1.0 - ssim_mean)
    return np.float32(loss)


# revision 10
# speedup vs baseline: 1.5086x; 1.5086x over previous
"""Trainium2 Bass kernel for CompositeLoss (0.16*MSE + 0.84*(1-SSIM)).

Data-parallel over 8 cores (2 images x 3 channels = 6 maps each). Per core,
per channel:
  - X,Y loaded as [128, 4, 512] (row r = partition + 128*jb)
  - pre-pass: U=X+Y, V=X-Y on Pool; UU=U^2 on DVE; VV=V^2 (+MSE accum) on ACT
  - stage-1 H-conv (fp16 PE): per (map, wc) 10 matmuls (4 owned-region +
    3 straddle pairs) into 2-bank PSUM pairs; evacuated to fp8 y1 via
    paired [128,2,502] copies split across ACT/DVE
  - stage-2 W-conv in fp8 DoubleRow (2 k-tiles per instr, 0.5 cyc/row):
    psum pairs [a|b], [h1|h2]
  - post-pass: P,Q = ACT square pair; B,A = Pool; n1d1 = DVE tensor_scalar;
    n2'd2' = DVE STT vs PSUM; NN,DD = DVE TT; recip on ACT; final
    accumulation on Pool STT
  - per-partition partial sums DMA'd out; host reduces + combines cores.
"""

import os
import sys

import numpy as np

sys.path.insert(0, "/opt/trn_rl_repo")

H = W = 512
OUT = 502
WIN = 11
SIG = 1.5
C1 = 0.01 ** 2
C2 = 0.03 ** 2
TC1 = float(2.0 * C1)
TC2 = float(2.0 * C2)
ALPHA = 0.84
NCH = 6
NCORES = 8
NACC = 32     # acc cols: 0..5 mse per ch, 8+ch*4+c ssim
SS0 = 8
OWN_OFF = [0, 128, 256, 384]


def _taps():
    c = np.arange(WIN, dtype=np.float64) - (WIN - 1) / 2.0
    g = np.exp(-(c ** 2) / (2.0 * SIG ** 2))
    g = g / g.sum()
    g16 = g.astype(np.float16).astype(np.float64)
    g16[5] = 1.0 - (g16.sum() - g16[5])
    g16 = g16.astype(np.float16).astype(np.float64)
    return g16


def _consts():
    import ml_dtypes
    g = _taps()
    f16 = np.float16
    fp8 = ml_dtypes.float8_e4m3

    band_own = np.zeros((128, 118), dtype=np.float64)
    for t in range(118):
        band_own[t:t + WIN, t] = g
    band_tail = np.zeros((128, 10), dtype=np.float64)
    for tl in range(10):
        for r in range(118 + tl, 128):
            band_tail[r, tl] = g[r - 118 - tl]
    band_head = np.zeros((128, 10), dtype=np.float64)
    for tl in range(10):
        for r in range(0, tl + 1):
            band_head[r, tl] = g[r + 10 - tl]

    bw = np.zeros((128, 128), dtype=np.float64)
    for m in range(128):
        k = np.arange(m, min(m + WIN, 128))
        bw[k, m] = g[k - m]
    bwh = np.zeros((128, 128), dtype=np.float64)
    for m in range(118, 128):
        k = np.arange(0, m - 118 + 1)
        bwh[k, m] = g[k + 128 - m]

    def renorm(mats, colsets):
        # nudge fp8 taps by one ulp each until every output column's tap
        # sum is 1 -- fp8 tap-sum error otherwise biases sigma estimates
        for locs in colsets:
            for _ in range(24):
                s = sum(float(mats[mi][r, c]) for mi, r, c in locs)
                err = 1.0 - s
                if abs(err) < 1e-7:
                    break
                best = None
                for mi, r, c in locs:
                    u = mats[mi][r, c].view(np.uint8)
                    for nb in (np.uint8(u + 1), np.uint8(u - 1)):
                        nv = nb.view(fp8)
                        nerr = abs(err - (float(nv) - float(mats[mi][r, c])))
                        if nerr < abs(err) - 1e-12 and (
                                best is None or nerr < best[0]):
                            best = (nerr, mi, r, c, nv)
                if best is None:
                    break
                _, mi, r, c, nv = best
                mats[mi][r, c] = nv
        return mats

    bw8, bwh8 = renorm(
        [bw.astype(fp8), bwh.astype(fp8)],
        [[(0, k, m) for k in range(m, min(m + WIN, 128))]
         + [(1, k, m) for k in range(0, max(0, m - 117))]
         for m in range(128)])
    bw118_8, = renorm(
        [bw[:, :118].astype(fp8)],
        [[(0, k, m) for k in range(m, m + WIN)] for m in range(118)])
    bwp8 = np.stack([bw8, bwh8], axis=1)       # [128, 2, 128]
    neg = lambda a: (a.view(np.uint8) ^ np.uint8(0x80)).view(fp8)

    return {
        "band_own": band_own.astype(f16),
        "band_tail": band_tail.astype(f16),
        "band_head": band_head.astype(f16),
        "bwp": bwp8,
        "bwpn": neg(bwp8),
        "bw118": bw118_8,
        "bw118n": neg(bw118_8),
    }


_NC_CACHE = {}

# evacuation engine per (map_idx, pair): "A" = ACT, "D" = DVE
EVAC = ["A", "D", "A", "A", "A", "D", "A", "A"]
# reciprocal engine per chunk c
RECIP = ["A", "D", "A", "D"]


def _build_nc():
    if "nc" in _NC_CACHE:
        return _NC_CACHE["nc"]
    from concourse import bass, bacc, mybir
    from concourse.tile import TileContext
    dt = mybir.dt
    AF = mybir.ActivationFunctionType
    OP = mybir.AluOpType
    DR = mybir.MatmulPerfMode.DoubleRow

    nc = bacc.Bacc(None, target_bir_lowering=False)
    pred = nc.dram_tensor("pred", [NCH, H, W], dt.float32, kind="ExternalInput")
    targ = nc.dram_tensor("targ", [NCH, H, W], dt.float32, kind="ExternalInput")
    bown_d = nc.dram_tensor("band_own", [128, 118], dt.float16, kind="ExternalInput")
    btail_d = nc.dram_tensor("band_tail", [128, 10], dt.float16, kind="ExternalInput")
    bhead_d = nc.dram_tensor("band_head", [128, 10], dt.float16, kind="ExternalInput")
    bwp_d = nc.dram_tensor("bwp", [128, 2, 128], dt.float8e4, kind="ExternalInput")
    bwpn_d = nc.dram_tensor("bwpn", [128, 2, 128], dt.float8e4, kind="ExternalInput")
    bw118_d = nc.dram_tensor("bw118", [128, 118], dt.float8e4, kind="ExternalInput")
    bw118n_d = nc.dram_tensor("bw118n", [128, 118], dt.float8e4, kind="ExternalInput")
    out_d = nc.dram_tensor("out_acc", [128, NACC], dt.float32, kind="ExternalOutput")

    with TileContext(nc) as tc:
        with (
            tc.tile_pool(name="const", bufs=1) as constp,
            tc.tile_pool(name="io", bufs=2) as iop,
            tc.tile_pool(name="maps", bufs=2) as mapp,
            tc.tile_pool(name="y1", bufs=2) as y1p,
            tc.tile_pool(name="post", bufs=2) as postp,
            tc.tile_pool(name="acc", bufs=1) as accp,
            tc.tile_pool(name="ps1", bufs=2, space="PSUM") as ps1p,
            tc.tile_pool(name="ps2", bufs=1, space="PSUM") as ps2p,
        ):
            bown = constp.tile([128, 118], dt.float16, name="bown")
            btail = constp.tile([128, 10], dt.float16, name="btail")
            bhead = constp.tile([128, 10], dt.float16, name="bhead")
            bwp = constp.tile([128, 2, 128], dt.float8e4, name="bwp")
            bwpn = constp.tile([128, 2, 128], dt.float8e4, name="bwpn")
            bw118 = constp.tile([128, 118], dt.float8e4, name="bw118")
            bw118n = constp.tile([128, 118], dt.float8e4, name="bw118n")
            nc.sync.dma_start(out=bown[:], in_=bown_d[:])
            nc.sync.dma_start(out=btail[:], in_=btail_d[:])
            nc.sync.dma_start(out=bhead[:], in_=bhead_d[:])
            nc.sync.dma_start(out=bwp[:], in_=bwp_d[:])
            nc.sync.dma_start(out=bwpn[:], in_=bwpn_d[:])
            nc.sync.dma_start(out=bw118[:], in_=bw118_d[:])
            nc.sync.dma_start(out=bw118n[:], in_=bw118n_d[:])

            acc = accp.tile([128, NACC], dt.float32, name="acc")
            nc.vector.memset(acc[:], 0.0)

            for ch in range(NCH):
                # ---- loads: [128, 4, 512], row r = p + 128*jb ----
                xt = iop.tile([128, 4, 512], dt.float32, tag="x", name=f"x{ch}")
                yt = iop.tile([128, 4, 512], dt.float32, tag="y", name=f"y{ch}")
                for jb in range(4):
                    r0 = 128 * jb
                    nc.sync.dma_start(out=xt[:, jb, :],
                                      in_=pred[ch, r0:r0 + 128, :])
                    nc.sync.dma_start(out=yt[:, jb, :],
                                      in_=targ[ch, r0:r0 + 128, :])

                # ---- pre-pass ----
                ut = mapp.tile([128, 4, 512], dt.float16, tag="u", name=f"u{ch}")
                vt = mapp.tile([128, 4, 512], dt.float16, tag="v", name=f"v{ch}")
                uut = mapp.tile([128, 4, 512], dt.float16, tag="uu", name=f"uu{ch}")
                vvt = mapp.tile([128, 4, 512], dt.float16, tag="vv", name=f"vv{ch}")
                nc.gpsimd.tensor_tensor(ut[:], xt[:], yt[:], OP.add)
                nc.gpsimd.tensor_tensor(vt[:], xt[:], yt[:], OP.subtract)
                nc.gpsimd.tensor_tensor(uut[:], ut[:], ut[:], OP.mult)
                # MSE = sum((X-Y)^2) rides the VV square
                nc.scalar.activation(vvt[:], vt[:], AF.Square,
                                     accum_out=acc[:, ch:ch + 1])

                maps_ = (ut, vt, uut, vvt)

                # ---- stage 1: H-conv (fp16) -> fp8 y1 [128, 16, 502] ----
                y1 = y1p.tile([128, 16, 502], dt.float8e4, tag="y1",
                              name=f"y1_{ch}")
                for mi in range(4):
                    mt = maps_[mi]
                    for pair in range(2):
                        ps = ps1p.tile([128, 2, 512], dt.float32, tag="ps1",
                                       name=f"ps1_{ch}_{mi}{pair}")
                        for half in range(2):
                            wc = pair * 2 + half
                            ws = slice(wc * 128, (wc + 1) * 128)
                            seq = []
                            for jb in range(4):
                                seq.append((mt[:, jb, ws], bown,
                                            OWN_OFF[jb], 118))
                            for b in (1, 2, 3):
                                seq.append((mt[:, b - 1, ws], btail,
                                            128 * b - 10, 10))
                                seq.append((mt[:, b, ws], bhead,
                                            128 * b - 10, 10))
                            n_mm = len(seq)
                            for i, (lhs, band, lo, n) in enumerate(seq):
                                nc.tensor.matmul(
                                    ps[:, half, lo:lo + n],
                                    lhsT=lhs, rhs=band[:, 0:n],
                                    start=(i == 0), stop=(i == n_mm - 1),
                                    skip_group_check=True)
                        dst = y1[:, mi * 4 + pair * 2: mi * 4 + pair * 2 + 2, :]
                        if EVAC[mi * 2 + pair] == "A":
                            nc.scalar.copy(dst, ps[:, :, 0:502])
                        else:
                            nc.vector.tensor_copy(dst, ps[:, :, 0:502])

                # ---- stage 2 (fp8 DoubleRow W-conv) + post-pass per c ----
                for c in range(4):
                    P = 128 if c < 3 else 118
                    ab = ps2p.tile([128, 2, 512], dt.float32, tag="ab",
                                   name=f"ab{ch}{c}")
                    hh = ps2p.tile([128, 2, 512], dt.float32, tag="hh",
                                   name=f"hh{ch}{c}")
                    if c < 3:
                        pr = lambda mi: y1[:, mi * 4 + c: mi * 4 + c + 2, :]
                        nc.tensor.matmul(ab[:, 0, 0:502], lhsT=bwp[:],
                                         rhs=pr(0), start=True, stop=True,
                                         perf_mode=DR, skip_group_check=True)
                        nc.tensor.matmul(ab[:, 1, 0:502], lhsT=bwp[:],
                                         rhs=pr(1), start=True, stop=True,
                                         perf_mode=DR, skip_group_check=True)
                        nc.tensor.matmul(hh[:, 0, 0:502], lhsT=bwp[:],
                                         rhs=pr(2), start=True, stop=False,
                                         perf_mode=DR, skip_group_check=True)
                        nc.tensor.matmul(hh[:, 0, 0:502], lhsT=bwpn[:],
                                         rhs=pr(3), start=False, stop=True,
                                         perf_mode=DR, skip_group_check=True)
                        nc.tensor.matmul(hh[:, 1, 0:502], lhsT=bwp[:],
                                         rhs=pr(2), start=True, stop=False,
                                         perf_mode=DR, skip_group_check=True)
                        nc.tensor.matmul(hh[:, 1, 0:502], lhsT=bwp[:],
                                         rhs=pr(3), start=False, stop=True,
                                         perf_mode=DR, skip_group_check=True)
                    else:
                        sl = lambda mi: y1[:, mi * 4 + 3, :]
                        nc.tensor.matmul(ab[0:118, 0, 0:502], lhsT=bw118[:],
                                         rhs=sl(0), start=True, stop=True,
                                         skip_group_check=True)
                        nc.tensor.matmul(ab[0:118, 1, 0:502], lhsT=bw118[:],
                                         rhs=sl(1), start=True, stop=True,
                                         skip_group_check=True)
                        nc.tensor.matmul(hh[0:118, 0, 0:502], lhsT=bw118[:],
                                         rhs=sl(2), start=True, stop=False,
                                         skip_group_check=True)
                        nc.tensor.matmul(hh[0:118, 0, 0:502], lhsT=bw118n[:],
                                         rhs=sl(3), start=False, stop=True,
                                         skip_group_check=True)
                        nc.tensor.matmul(hh[0:118, 1, 0:502], lhsT=bw118[:],
                                         rhs=sl(2), start=True, stop=False,
                                         skip_group_check=True)
                        nc.tensor.matmul(hh[0:118, 1, 0:502], lhsT=bw118[:],
                                         rhs=sl(3), start=False, stop=True,
                                         skip_group_check=True)

                    # post-pass
                    pq = postp.tile([128, 1004], dt.float16, tag="pq",
                                    name=f"pq{ch}{c}")
                    ba = postp.tile([128, 1004], dt.float16, tag="ba",
                                    name=f"ba{ch}{c}")
                    nd = postp.tile([128, 1004], dt.float16, tag="nd",
                                    name=f"nd{ch}{c}")
                    n2d2 = postp.tile([128, 1004], dt.float16, tag="n2d2",
                                      name=f"n2d2{ch}{c}")
                    nndd = postp.tile([128, 1004], dt.float16, tag="nndd",
                                      name=f"nndd{ch}{c}")
                    rr = postp.tile([128, 502], dt.float16, tag="rr",
                                    name=f"rr{ch}{c}")
                    junk = postp.tile([128, 502], dt.float16, tag="junk",
                                      name=f"junk{ch}{c}")

                    # P|Q = square(a|b)
                    nc.scalar.activation(pq[0:P, :], ab[0:P, :, 0:502],
                                         AF.Square)
                    # B = P-Q ; A = P+Q   (Pool)
                    nc.gpsimd.tensor_tensor(ba[0:P, 0:502], pq[0:P, 0:502],
                                            pq[0:P, 502:1004], OP.subtract)
                    nc.gpsimd.tensor_tensor(ba[0:P, 502:1004], pq[0:P, 0:502],
                                            pq[0:P, 502:1004], OP.add)
                    # n1|d1 = (B|A) + 2C1
                    nc.vector.tensor_scalar_add(nd[0:P, :], ba[0:P, :], TC1)
                    # n2'|d2' = (B|A - 2C2) - (h1|h2)   [negated n2, d2]
                    nc.vector.scalar_tensor_tensor(
                        n2d2[0:P, :], ba[0:P, :], TC2, hh[0:P, :, 0:502],
                        OP.subtract, OP.subtract)
                    # NN|DD  (Pool)
                    nc.gpsimd.tensor_tensor(nndd[0:P, :], nd[0:P, :],
                                            n2d2[0:P, :], OP.mult)
                    with nc.allow_low_precision(reason="fp16 ssim recip"):
                        nc.vector.reciprocal(rr[0:P, :],
                                             nndd[0:P, 502:1004])
                        sidx = SS0 + ch * 4 + c
                        nc.vector.scalar_tensor_tensor(
                            junk[0:P, :], nndd[0:P, 0:502], 1.0, rr[0:P, :],
                            OP.mult, OP.mult,
                            accum_out=acc[0:P, sidx:sidx + 1])

            nc.sync.dma_start(out=out_d[:], in_=acc[:])

    nc.compile()
    _NC_CACHE["nc"] = nc
    return nc


def kernel(pred: np.ndarray, target: np.ndarray) -> np.ndarray:
    from concourse.bass_utils import run_bass_kernel_spmd

    pred = np.asarray(pred, dtype=np.float32)
    target = np.asarray(target, dtype=np.float32)
    cst = _consts()

    nc = _build_nc()
    in_maps = []
    for i in range(NCORES):
        m = {
            "pred": pred[2 * i:2 * i + 2].reshape(NCH, H, W),
            "targ": target[2 * i:2 * i + 2].reshape(NCH, H, W),
        }
        m.update(cst)
        in_maps.append(m)

    trace = os.environ.get("BASS_SSIM_TRACE", "0") == "1"
    res = run_bass_kernel_spmd(nc, in_maps, core_ids=list(range(NCORES)),
                               trace=trace)
    if trace and res.exec_time_ns is not None:
        print(f"HW exec time: {res.exec_time_ns} ns")
        _NC_CACHE["exec_time_ns"] = res.exec_time_ns

    mse_sum = 0.0
    ssim_sum = 0.0
    for i in range(NCORES):
        o = np.asarray(res.results[i]["out_acc"], dtype=np.float64)
        mse_sum += float(o[:, 0:NCH].sum())
        ssim_sum += float(o[:, SS0:SS0 + NCH * 4].sum())

    mse_mean = mse_sum / (16 * 3 * H * W)
    ssim_mean = ssim_sum / (16 * 3 * OUT * OUT)
    loss = (1.0 - ALPHA) * mse_mean + ALPHA * (1.0 - ssim_mean)
    return np.float32(loss)
